# revision 6
# baseline (speedup 1.0000x reference)
"""Single-head attention (B=4, T=4096, C=120, H=64) on 8 Trainium2 cores.

Sharding: data-parallel over (batch, query-half). Core c handles batch c//2,
query rows [(c%2)*2048, (c%2+1)*2048). Each core receives x[b].T with the
sequence axis rolled so its query block sits at columns 0..2047 (softmax
attention is permutation-invariant along the key axis, so rolling K/V is
harmless), computes K/Q/V projections and the full attention for its 2048
query rows, and returns O^T [64, 2048]. The host transposes and concatenates.

On-device layout (all matmuls in float32r = fp22 multiply, fp32 accumulate):
  xT_sb  [120, 4096]  x^T staged in SBUF
  KT_sb  [128, 4096]  K^T duplicated on partitions 0-63 / 64-127 (for 2x
                      row-packed S^T matmuls using both halves of the PE)
  QT_sb  [128, 2048]  Q^T duplicated likewise (Wq pre-scaled by C^-0.5)
  Vp_sb  [128, 32, 65] V tiles [s-tile, 65] with a ones column at index 64,
                      so the PV matmul also accumulates the softmax
                      denominator into psum_o row 64.
  S^T tiles [128 s, 512 tq] in PSUM -> exp on ScalarE (grouped 4/3 tiles per
  ACTIVATE to amortize the ~352-cycle instruction overhead) -> SBUF ->
  PV accumulation into psum_o [65, 512] -> reciprocal + broadcast + multiply.
"""

import numpy as np

import concourse.bass as bass
import concourse.bacc as bacc
import concourse.tile as tile
from concourse import mybir
from concourse.bass_utils import run_bass_kernel_spmd

B, T, C, H = 4, 4096, 120, 64
NCORES = 8
TQ = T // 2            # query rows per core
QCHUNK = 512           # moving-dim width per matmul / psum bank
NQC = TQ // QCHUNK     # 4 query chunks
ST = 128               # s-tile height (partition dim of S^T)
NST = T // ST          # 32 s-tiles
# 9 exp groups per query chunk: five of 4 s-tiles, four of 3 (5*4+4*3=32).
GROUPS = [(0, 4), (4, 3), (7, 4), (11, 3), (14, 4), (18, 3), (21, 4), (25, 3), (28, 4)]

F32 = mybir.dt.float32
F32R = mybir.dt.float32r
EXP = mybir.ActivationFunctionType.Exp

PACK_ROWS = False      # fp32r matmuls reject tile_position offsets (ISA check)


def _build():
    nc = bacc.Bacc("TRN2", target_bir_lowering=False, debug=False)
    xT = nc.dram_tensor("xT", [C, T], F32, kind="ExternalInput").ap()
    wk = nc.dram_tensor("wk", [C, H], F32, kind="ExternalInput").ap()
    wq = nc.dram_tensor("wq", [C, H], F32, kind="ExternalInput").ap()
    wv = nc.dram_tensor("wv", [C, H], F32, kind="ExternalInput").ap()
    ones = nc.dram_tensor("ones", [128, NST], F32, kind="ExternalInput").ap()
    outT = nc.dram_tensor("outT", [H, TQ], F32, kind="ExternalOutput").ap()

    from contextlib import ExitStack
    with tile.TileContext(nc) as tc, ExitStack() as ctx:
        consts = ctx.enter_context(tc.tile_pool(name="consts", bufs=1))
        bigs = ctx.enter_context(tc.tile_pool(name="bigs", bufs=1))
        exps = ctx.enter_context(tc.tile_pool(name="exps", bufs=1))
        small = ctx.enter_context(tc.tile_pool(name="small", bufs=2))
        dram = ctx.enter_context(tc.tile_pool(name="dram", bufs=2, space="DRAM"))

        # ACT exp-table preload overlapping the input DMA.
        dummy = consts.tile([1, 1], F32)
        nc.vector.memset(dummy[:], 0.0)
        nc.scalar.activation(out=dummy[:], in_=dummy[:], func=EXP)

        wk_sb = consts.tile([C, H], F32R)
        wq_sb = consts.tile([C, H], F32R)
        wv_sb = consts.tile([C, H], F32R)
        nc.sync.dma_start(out=wk_sb[:], in_=wk.bitcast(F32R))
        nc.sync.dma_start(out=wq_sb[:], in_=wq.bitcast(F32R))
        nc.sync.dma_start(out=wv_sb[:], in_=wv.bitcast(F32R))

        xT_sb = bigs.tile([C, T], F32R)
        for j in range(T // QCHUNK):
            sl = slice(j * QCHUNK, (j + 1) * QCHUNK)
            nc.sync.dma_start(out=xT_sb[:, sl], in_=xT[:, sl].bitcast(F32R))

        KP = 128 if PACK_ROWS else 64
        KT_sb = bigs.tile([KP, T], F32R)
        QT_sb = bigs.tile([KP, TQ], F32R)
        Vp_sb = bigs.tile([128, NST, H + 1], F32R)
        nc.sync.dma_start(out=Vp_sb[:, :, H], in_=ones.bitcast(F32R))

        with tc.tile_pool(name="pp_proj", bufs=2, space="PSUM") as pp:
            # K^T / Q^T: weight-stationary, col-packed twin matmuls write the
            # duplicate copy into psum partitions 64-127 nearly for free.
            for w_sb, dst_sb, nchunks in ((wk_sb, KT_sb, T // QCHUNK),
                                          (wq_sb, QT_sb, NQC)):
                for j in range(nchunks):
                    sl = slice(j * QCHUNK, (j + 1) * QCHUNK)
                    kq = pp.tile([64, QCHUNK], F32, tag="kq")
                    nc.tensor.matmul(kq[:], w_sb[:], xT_sb[:, sl],
                                     start=True, stop=True)
                    nc.vector.tensor_copy(out=dst_sb[0:64, sl], in_=kq[:])
                    if PACK_ROWS:
                        nc.sync.dma_start(out=dst_sb[64:128, sl],
                                          in_=kq[:].bitcast(F32R))
            # V: x-stationary per s-tile, N=64 moving.
            for st in range(NST):
                pv = pp.tile([128, H], F32, tag="pv")
                nc.tensor.matmul(pv[:], xT_sb[:, st * ST:(st + 1) * ST],
                                 wv_sb[:], start=True, stop=True)
                nc.vector.tensor_copy(out=Vp_sb[:, st, 0:H], in_=pv[:])

        with (tc.tile_pool(name="ppA", bufs=1, space="PSUM") as ppA,
              tc.tile_pool(name="ppB", bufs=1, space="PSUM") as ppB,
              tc.tile_pool(name="ppO", bufs=1, space="PSUM") as ppO):
            for qc in range(NQC):
                qsl = slice(qc * QCHUNK, (qc + 1) * QCHUNK)
                psum_o = ppO.tile([H + 1, QCHUNK], F32, tag="o")
                pending = []

                def emit_pv(expX, st0, sz):
                    for i in range(sz):
                        st = st0 + i
                        nc.tensor.matmul(
                            psum_o[:],
                            Vp_sb[:, st, :],
                            expX[:, i * QCHUNK:(i + 1) * QCHUNK],
                            start=(st == 0), stop=(st == NST - 1))

                for st0, sz in GROUPS:
                    if sz == 4:
                        ps = ppA.tile([128, 4 * QCHUNK], F32, tag="sA")
                        expX = exps.tile([128, 4 * QCHUNK], F32R, tag="expA")
                    else:
                        ps = ppB.tile([128, 3 * QCHUNK], F32, tag="sB")
                        expX = exps.tile([128, 3 * QCHUNK], F32R, tag="expB")
                    i = 0
                    while i < sz:
                        st = st0 + i
                        ksl = slice(st * ST, (st + 1) * ST)
                        osl = slice(i * QCHUNK, (i + 1) * QCHUNK)
                        if PACK_ROWS and i + 1 < sz:
                            ksl2 = slice((st + 1) * ST, (st + 2) * ST)
                            osl2 = slice((i + 1) * QCHUNK, (i + 2) * QCHUNK)
                            nc.tensor.matmul(ps[:, osl],
                                             KT_sb[0:64, ksl],
                                             QT_sb[0:64, qsl],
                                             start=True, stop=True)
                            nc.tensor.matmul(ps[:, osl2],
                                             KT_sb[64:128, ksl2],
                                             QT_sb[64:128, qsl],
                                             start=True, stop=True)
                            i += 2
                        else:
                            nc.tensor.matmul(ps[:, osl],
                                             KT_sb[0:64, ksl],
                                             QT_sb[0:64, qsl],
                                             start=True, stop=True)
                            i += 1
                    nc.scalar.activation(out=expX[:, 0:sz * QCHUNK],
                                         in_=ps[:, 0:sz * QCHUNK], func=EXP)
                    pending.append((expX, st0, sz))
                    if len(pending) > 1:
                        emit_pv(*pending.pop(0))
                emit_pv(*pending.pop(0))

                # normalize: rows 0-63 are the numerator, row 64 the denominator
                r = small.tile([H + 1, QCHUNK], F32, tag="r")
                nc.vector.reciprocal(out=r[H:H + 1, :], in_=psum_o[H:H + 1, :])
                scr = dram.tile([1, QCHUNK], F32, tag="scr")
                nc.sync.dma_start(out=scr[:], in_=r[H:H + 1, :])
                scr_ap = scr[:]
                rbc = bass.AP(tensor=scr_ap.tensor, offset=scr_ap.offset,
                              ap=[[0, H], [1, QCHUNK]])
                rb = small.tile([H, QCHUNK], F32, tag="rb")
                nc.sync.dma_start(out=rb[:], in_=rbc)
                outc = small.tile([H, QCHUNK], F32, tag="outc")
                nc.vector.tensor_mul(outc[:], psum_o[0:H, :], rb[:])
                nc.sync.dma_start(out=outT[:, qsl], in_=outc[:])

    nc.compile()
    return nc


_NC = None


def _get_nc():
    global _NC
    if _NC is None:
        _NC = _build()
    return _NC


def _make_in_maps(x, Wk, Wq, Wv):
    wqs = (Wq.astype(np.float64) * (C ** -0.5)).astype(np.float32)
    wk = np.ascontiguousarray(Wk, dtype=np.float32)
    wv = np.ascontiguousarray(Wv, dtype=np.float32)
    in_maps = []
    for core in range(NCORES):
        b, qh = core // 2, core % 2
        xr = np.roll(np.asarray(x[b], dtype=np.float32), -qh * TQ, axis=0)
        in_maps.append({
            "xT": np.ascontiguousarray(xr.T),
            "wk": wk, "wq": wqs, "wv": wv,
            "ones": np.ones((128, NST), np.float32),
        })
    return in_maps


def _assemble(results):
    out = np.empty((B, T, H), np.float64)
    for core in range(NCORES):
        b, qh = core // 2, core % 2
        out[b, qh * TQ:(qh + 1) * TQ, :] = results[core]["outT"].T.astype(np.float64)
    return out


def kernel(x, Wk, Wq, Wv):
    nc = _get_nc()
    res = run_bass_kernel_spmd(nc, _make_in_maps(x, Wk, Wq, Wv),
                               list(range(NCORES)))
    return _assemble(res.results)


# revision 8
# speedup vs baseline: 1.2296x; 1.2296x over previous
"""Single-head attention (B=4, T=4096, C=120, H=64) on 8 Trainium2 cores.

Sharding: data-parallel over (batch, query-half). Core c handles batch c//2,
query rows [(c%2)*2048, (c%2+1)*2048). Each core receives x[b].T with the
sequence axis rolled so its query block sits at columns 0..2047 (softmax
attention is permutation-invariant along the key axis, so rolling K/V is
harmless), computes K/Q/V projections and the full attention for its 2048
query rows, and returns O^T [64, 2048]. The host transposes and concatenates.

On-device layout (all matmuls in float32r = fp22 multiply, fp32 accumulate):
  xT_sb  [120, 4096]  x^T staged in SBUF
  KT_sb  [128, 4096]  K^T duplicated on partitions 0-63 / 64-127 (for 2x
                      row-packed S^T matmuls using both halves of the PE)
  QT_sb  [128, 2048]  Q^T duplicated likewise (Wq pre-scaled by C^-0.5)
  Vp_sb  [128, 32, 65] V tiles [s-tile, 65] with a ones column at index 64,
                      so the PV matmul also accumulates the softmax
                      denominator into psum_o row 64.
  S^T tiles [128 s, 512 tq] in PSUM -> exp on ScalarE (grouped 4/3 tiles per
  ACTIVATE to amortize the ~352-cycle instruction overhead) -> SBUF ->
  PV accumulation into psum_o [65, 512] -> reciprocal + broadcast + multiply.
"""

import numpy as np

import concourse.bass as bass
import concourse.bacc as bacc
import concourse.tile as tile
from concourse import mybir
from concourse.bass_utils import run_bass_kernel_spmd

B, T, C, H = 4, 4096, 120, 64
NCORES = 8
TQ = T // 2            # query rows per core
QCHUNK = 512           # moving-dim width per matmul / psum bank
NQC = TQ // QCHUNK     # 4 query chunks
ST = 128               # s-tile height (partition dim of S^T)
NST = T // ST          # 32 s-tiles
# 9 exp groups per query chunk: five of 4 s-tiles, four of 3 (5*4+4*3=32).
GROUPS = [(0, 4), (4, 3), (7, 4), (11, 3), (14, 4), (18, 3), (21, 4), (25, 3), (28, 4)]

F32 = mybir.dt.float32
F32R = mybir.dt.float32r
F16 = mybir.dt.float16
EXP = mybir.ActivationFunctionType.Exp

PACK_ROWS = True       # fp16 S^T matmuls row-packed on PE rows 0-63 / 64-127


def _build():
    nc = bacc.Bacc("TRN2", target_bir_lowering=False, debug=False)
    xT = nc.dram_tensor("xT", [C, T], F32, kind="ExternalInput").ap()
    wk = nc.dram_tensor("wk", [C, H], F32, kind="ExternalInput").ap()
    wq = nc.dram_tensor("wq", [C, H], F32, kind="ExternalInput").ap()
    wv = nc.dram_tensor("wv", [C, H], F32, kind="ExternalInput").ap()
    ones = nc.dram_tensor("ones", [128, NST], F16, kind="ExternalInput").ap()
    outT = nc.dram_tensor("outT", [H, TQ], F32, kind="ExternalOutput").ap()

    from contextlib import ExitStack
    with tile.TileContext(nc) as tc, ExitStack() as ctx:
        consts = ctx.enter_context(tc.tile_pool(name="consts", bufs=1))
        bigs = ctx.enter_context(tc.tile_pool(name="bigs", bufs=1))
        exps = ctx.enter_context(tc.tile_pool(name="exps", bufs=1))
        small = ctx.enter_context(tc.tile_pool(name="small", bufs=2))
        dram = ctx.enter_context(tc.tile_pool(name="dram", bufs=2, space="DRAM"))

        # ACT exp-table preload overlapping the input DMA.
        dummy = consts.tile([1, 1], F32)
        nc.vector.memset(dummy[:], 0.0)
        nc.scalar.activation(out=dummy[:], in_=dummy[:], func=EXP)

        wk_sb = consts.tile([C, H], F32R)
        wq_sb = consts.tile([C, H], F32R)
        wv_sb = consts.tile([C, H], F32R)
        nc.sync.dma_start(out=wk_sb[:], in_=wk.bitcast(F32R))
        nc.sync.dma_start(out=wq_sb[:], in_=wq.bitcast(F32R))
        nc.sync.dma_start(out=wv_sb[:], in_=wv.bitcast(F32R))

        xT_sb = bigs.tile([C, T], F32R)
        for j in range(T // QCHUNK):
            sl = slice(j * QCHUNK, (j + 1) * QCHUNK)
            nc.sync.dma_start(out=xT_sb[:, sl], in_=xT[:, sl].bitcast(F32R))

        KP = 128 if PACK_ROWS else 64
        KT_sb = bigs.tile([KP, T], F16)
        QT_sb = bigs.tile([KP, TQ], F16)
        Vp_sb = bigs.tile([128, NST, H + 1], F16)
        nc.sync.dma_start(out=Vp_sb[:, :, H], in_=ones)

        with tc.tile_pool(name="pp_proj", bufs=2, space="PSUM") as pp:
            # K^T / Q^T: weight-stationary, col-packed twin matmuls write the
            # duplicate copy into psum partitions 64-127 nearly for free.
            for w_sb, dst_sb, nchunks in ((wk_sb, KT_sb, T // QCHUNK),
                                          (wq_sb, QT_sb, NQC)):
                for j in range(nchunks):
                    sl = slice(j * QCHUNK, (j + 1) * QCHUNK)
                    kq = pp.tile([64, QCHUNK], F32, tag="kq")
                    nc.tensor.matmul(kq[:], w_sb[:], xT_sb[:, sl],
                                     start=True, stop=True)
                    nc.vector.tensor_copy(out=dst_sb[0:64, sl], in_=kq[:])
                    if PACK_ROWS:
                        nc.sync.dma_start(out=dst_sb[64:128, sl],
                                          in_=dst_sb[0:64, sl])
            # V: x-stationary per s-tile, N=64 moving.
            for st in range(NST):
                pv = pp.tile([128, H], F32, tag="pv")
                nc.tensor.matmul(pv[:], xT_sb[:, st * ST:(st + 1) * ST],
                                 wv_sb[:], start=True, stop=True)
                nc.vector.tensor_copy(out=Vp_sb[:, st, 0:H], in_=pv[:])

        with (tc.tile_pool(name="ppA", bufs=1, space="PSUM") as ppA,
              tc.tile_pool(name="ppB", bufs=1, space="PSUM") as ppB,
              tc.tile_pool(name="ppO", bufs=1, space="PSUM") as ppO):
            for qc in range(NQC):
                qsl = slice(qc * QCHUNK, (qc + 1) * QCHUNK)
                psum_o = ppO.tile([H + 1, QCHUNK], F32, tag="o")
                pending = []

                def emit_pv(expX, st0, sz):
                    for i in range(sz):
                        st = st0 + i
                        nc.tensor.matmul(
                            psum_o[:],
                            Vp_sb[:, st, :],
                            expX[:, i * QCHUNK:(i + 1) * QCHUNK],
                            start=(st == 0), stop=(st == NST - 1))

                for st0, sz in GROUPS:
                    if sz == 4:
                        ps = ppA.tile([128, 4 * QCHUNK], F32, tag="sA")
                        expX = exps.tile([128, 4 * QCHUNK], F16, tag="expA")
                    else:
                        ps = ppB.tile([128, 3 * QCHUNK], F32, tag="sB")
                        expX = exps.tile([128, 3 * QCHUNK], F16, tag="expB")
                    i = 0
                    while i < sz:
                        st = st0 + i
                        ksl = slice(st * ST, (st + 1) * ST)
                        osl = slice(i * QCHUNK, (i + 1) * QCHUNK)
                        if PACK_ROWS and i + 1 < sz:
                            ksl2 = slice((st + 1) * ST, (st + 2) * ST)
                            osl2 = slice((i + 1) * QCHUNK, (i + 2) * QCHUNK)
                            nc.tensor.matmul(ps[:, osl],
                                             KT_sb[0:64, ksl],
                                             QT_sb[0:64, qsl],
                                             start=True, stop=True)
                            nc.tensor.matmul(ps[:, osl2],
                                             KT_sb[64:128, ksl2],
                                             QT_sb[64:128, qsl],
                                             start=True, stop=True)
                            i += 2
                        else:
                            nc.tensor.matmul(ps[:, osl],
                                             KT_sb[0:64, ksl],
                                             QT_sb[0:64, qsl],
                                             start=True, stop=True)
                            i += 1
                    nc.scalar.activation(out=expX[:, 0:sz * QCHUNK],
                                         in_=ps[:, 0:sz * QCHUNK], func=EXP)
                    pending.append((expX, st0, sz))
                    if len(pending) > 1:
                        emit_pv(*pending.pop(0))
                emit_pv(*pending.pop(0))

                # normalize: rows 0-63 are the numerator, row 64 the denominator
                r = small.tile([H + 1, QCHUNK], F32, tag="r")
                nc.vector.tensor_copy(out=r[H:H + 1, :], in_=psum_o[H:H + 1, :])
                scr = dram.tile([1, QCHUNK], F32, tag="scr")
                nc.sync.dma_start(out=scr[:], in_=r[H:H + 1, :])
                scr_ap = scr[:]
                rbc = bass.AP(tensor=scr_ap.tensor, offset=scr_ap.offset,
                              ap=[[0, H], [1, QCHUNK]])
                rb = small.tile([H, QCHUNK], F32, tag="rb")
                nc.sync.dma_start(out=rb[:], in_=rbc)
                nc.vector.reciprocal(out=rb[:], in_=rb[:])
                outc = small.tile([H, QCHUNK], F32, tag="outc")
                nc.vector.tensor_mul(outc[:], psum_o[0:H, :], rb[:])
                nc.sync.dma_start(out=outT[:, qsl], in_=outc[:])

    nc.compile()
    return nc


_NC = None


def _get_nc():
    global _NC
    if _NC is None:
        _NC = _build()
    return _NC


def _make_in_maps(x, Wk, Wq, Wv):
    wqs = (Wq.astype(np.float64) * (C ** -0.5)).astype(np.float32)
    wk = np.ascontiguousarray(Wk, dtype=np.float32)
    wv = np.ascontiguousarray(Wv, dtype=np.float32)
    in_maps = []
    for core in range(NCORES):
        b, qh = core // 2, core % 2
        xr = np.roll(np.asarray(x[b], dtype=np.float32), -qh * TQ, axis=0)
        in_maps.append({
            "xT": np.ascontiguousarray(xr.T),
            "wk": wk, "wq": wqs, "wv": wv,
            "ones": np.ones((128, NST), np.float16),
        })
    return in_maps


def _assemble(results):
    out = np.empty((B, T, H), np.float64)
    for core in range(NCORES):
        b, qh = core // 2, core % 2
        out[b, qh * TQ:(qh + 1) * TQ, :] = results[core]["outT"].T.astype(np.float64)
    return out


def kernel(x, Wk, Wq, Wv):
    nc = _get_nc()
    res = run_bass_kernel_spmd(nc, _make_in_maps(x, Wk, Wq, Wv),
                               list(range(NCORES)))
    return _assemble(res.results)
